# revision 16
# baseline (speedup 1.0000x reference)
"""Trainium2 Bass kernel for nn_MemConLoss_trans (supervised-contrastive loss
with memory-bank hard negatives).

Strategy (8 NeuronCores, SPMD, no collectives):
  - Grid-shard the score matrix: 4-way over B x 2-way over M. Core c handles
    score rows 256*(c%4) .. +256 against bank cols 32768*(c//4) .. +32768.
    Every tensor a core needs is staged from the host, so there is no
    AllGather (the baseline's collective cost ~96us of launch-skew barrier).
  - bank is host-pretransposed into the fp8 DoubleRow matmul layout
    [chunk, 128, 2, 2048]; score matmuls run in fp8 DoubleRow mode
    (2 rows/cycle, K=256 per instruction, ~157 TF/s when ramped).
  - box (s_box_feat) is host-cast to f16 with the h=48 column pre-folded
    (49 -> 48 spatial positions) and pretransposed to [d-tile, b, h]; the
    device reduces over h with a tensor_tensor pairwise fold tree (DVE 2x
    f16 mode, ~2x faster than tensor_reduce which has no fast mode), then
    the scalar engine casts to fp8 with the -4/49 mean+negate+range scale.
  - The -4*score matrix (8.4M f32 in PSUM per core) is scanned by both
    engines in parallel: DVE groups do a two-level tensor_tensor max fold
    (each tt reads two operands per cycle, so 2048 PSUM values cost ~1.3us)
    producing 512 slot-maxima shipped straight to DRAM; scalar groups do an
    Exp softmax-accumulator (one pass, accum_out) producing a soft max per
    2048 columns. Host merges both candidate kinds and takes the top-5
    smallest scores (their loss contribution is ~1e-6, so soft-max bias and
    fp8/f16 rounding are harmless).
  - The [B,B] contrastive logits are data-parallel over B (128 rows/core):
    host pre-normalizes and pre-transposes the operands, the device runs
    f32r matmuls and one exp row-sum on the scalar engine.
  - Host finalize: logits diagonal, candidate merge, final log/mean in f64.
    The constant shift 4.0 stands in for the per-row logits max (its only
    effect is ~1e-6 via exp(-MX)*negsum, as in the previous baseline).
"""

import numpy as np

B = 1024
D = 256
HWSP = 49              # 7*7 spatial positions
HF = 48                # h after host pre-fold of the odd column
NCORES = 8
GB = 4                 # B-shard factor of the score grid
GM = 2                 # M-shard factor
RB = B // GB           # 256 score rows per core
MC = 65536 // GM       # 32768 bank cols per core
NGRP = MC // 2048      # 16 psum groups per b-tile
NBT = RB // 128        # 2 b-tiles per core
MX = 4.0               # constant stand-in for the per-row logits max
TEMP = 0.07
QSC = 4.0              # fp8 range scale on the query
BETA = 2.0             # softmax sharpness on t' = -QSC*score
C1 = 40.0              # softmax shift on t'

# scan order: chunks in DMA-arrival order (sync evens, act odds, gpsimd tail)
SEQ = list(range(16))
# positions in SEQ consumed by the DVE slot-max lane; scalar takes the rest
DVE_POS = [{1, 3, 5, 7, 9, 11}, {1, 3, 5, 7, 9, 11, 13}]
DVE_GRPS = [sorted(SEQ[p] for p in DVE_POS[bt]) for bt in range(NBT)]
SCL_GRPS = [sorted(SEQ[p] for p in set(range(NGRP)) - DVE_POS[bt])
            for bt in range(NBT)]

_CACHE = {}


def _build_module():
    import concourse.bacc as bacc
    import concourse.mybir as mybir
    import concourse.tile as tile

    F32 = mybir.dt.float32
    F32R = mybir.dt.float32r
    F16 = mybir.dt.float16
    FP8 = mybir.dt.float8e4
    AF = mybir.ActivationFunctionType
    ALU = mybir.AluOpType
    X = mybir.AxisListType.X
    DR = mybir.MatmulPerfMode.DoubleRow

    nc = bacc.Bacc("TRN2", target_bir_lowering=False, debug=False,
                   enable_asserts=False, num_devices=NCORES)

    sqnT = nc.dram_tensor("sqnT", [2, 128, 128], F16, kind="ExternalInput").ap()
    msqnT = nc.dram_tensor("msqnT", [2, 128, B], F16, kind="ExternalInput").ap()
    boxT = nc.dram_tensor("boxT", [2, 128, RB, HF], FP8,
                          kind="ExternalInput").ap()
    bankT = nc.dram_tensor("bankT", [NGRP, 128, 2, 2048], FP8,
                           kind="ExternalInput").ap()
    o_rowsum = nc.dram_tensor("o_rowsum", [128, 1], F32,
                              kind="ExternalOutput").ap()
    o_slot = nc.dram_tensor("o_slot", [128, NBT * NGRP * 16], F16,
                            kind="ExternalOutput").ap()
    o_rs = nc.dram_tensor("o_rs", [128, NBT * NGRP], F32,
                          kind="ExternalOutput").ap()

    with nc.allow_low_precision("f16 box sums only feed the fp8 score matmul"):
      with tile.TileContext(nc) as tc:
        with (
            tc.tile_pool(name="main", bufs=1) as main,
            tc.tile_pool(name="scr", bufs=2) as scrp,
        ):
            sqnT_sb = main.tile([128, 2, 128], F16)
            msqnT_sb = main.tile([128, 2, B], F16)
            box_sb = main.tile([128, 2, RB, HF], F16)
            bankT_sb = main.tile([128, NGRP, 2, 2048], FP8)

            # ------------- input DMAs over 3 queues, arrival-ordered -------
            # act queue moves fastest early: give it box b-half 0 (the
            # critical path). gpsimd: logits operands then box q1 + odd tail.
            # sync: the bank even chunks. Late act DMAs are issued from
            # inside the scalar stream to dodge queue-credit blocking.
            Q = RB // 4  # 64-row box quarters

            def box_dma(t, q):
                # fp8 wire -> f16 SBUF cast dma (gpsimd swdge only)
                nc.gpsimd.dma_start(box_sb[:, t, q * Q:(q + 1) * Q, :],
                                    boxT[t, :, q * Q:(q + 1) * Q, :])

            box_dma(0, 0)
            box_dma(1, 0)
            nc.gpsimd.dma_start(msqnT_sb[:, 0], msqnT[0])
            nc.gpsimd.dma_start(msqnT_sb[:, 1], msqnT[1])
            nc.gpsimd.dma_start(sqnT_sb[:, 0], sqnT[0])
            nc.gpsimd.dma_start(sqnT_sb[:, 1], sqnT[1])
            box_dma(0, 1)
            box_dma(1, 1)
            box_dma(0, 2)
            box_dma(1, 2)
            box_dma(0, 3)
            box_dma(1, 3)
            nc.scalar.dma_start(bankT_sb[:, 1], bankT[1])
            nc.scalar.dma_start(bankT_sb[:, 3], bankT[3])
            for g in range(0, 16, 2):
                nc.sync.dma_start(bankT_sb[:, g], bankT[g])
            nc.sync.dma_start(bankT_sb[:, 13], bankT[13])
            nc.sync.dma_start(bankT_sb[:, 15], bankT[15])

            # ------------- box fold tree -> fp8 query ----------------------
            t24 = main.tile([128, 2, RB, 24], F16)
            t12 = main.tile([128, 2, RB, 12], F16)
            qsA = main.tile([128, 2, RB], F16)
            qsumT = main.tile([128, 2, RB], F16)
            nqT = main.tile([128, 2, RB], FP8)
            bias_mx = main.tile([128, 1], F32)
            nc.vector.memset(bias_mx[:], -MX)
            bias_sm = main.tile([128, 1], F32)
            nc.vector.memset(bias_sm[:], -BETA * C1)
            rs = main.tile([128, NBT, NGRP], F32)
            nc.vector.memset(rs[:], 0.0)
            slot = main.tile([128, NBT, NGRP, 16], F16)
            nc.vector.memset(slot[:], -1000.0)

            def box_casts(h):
                r = slice(h * 128, (h + 1) * 128)
                for t in range(2):
                    nc.scalar.activation(nqT[:, t, r], qsumT[:, t, r],
                                         AF.Copy, scale=-QSC / HWSP)

            def box_tree(h):  # h in {0,1}: rows h*128 .. h*128+128
                r = slice(h * 128, (h + 1) * 128)
                tt = nc.vector.tensor_tensor
                for q in (2 * h, 2 * h + 1):
                    s = slice(q * Q, (q + 1) * Q)
                    tt(out=t24[:, :, s, :], in0=box_sb[:, :, s, 0:24],
                       in1=box_sb[:, :, s, 24:48], op=ALU.add)
                tt(out=t12[:, :, r, :], in0=t24[:, :, r, 0:12],
                   in1=t24[:, :, r, 12:24], op=ALU.add)
                tt(out=t24[:, :, r, 0:6], in0=t12[:, :, r, 0:6],
                   in1=t12[:, :, r, 6:12], op=ALU.add)
                tt(out=t12[:, :, r, 0:3], in0=t24[:, :, r, 0:3],
                   in1=t24[:, :, r, 3:6], op=ALU.add)
                tt(out=qsA[:, :, r], in0=t12[:, :, r, 0],
                   in1=t12[:, :, r, 1], op=ALU.add)
                tt(out=qsumT[:, :, r], in0=qsA[:, :, r],
                   in1=t12[:, :, r, 2], op=ALU.add)

            box_tree(0)

            # ------------- logits: f32r matmul + exp rowsum ----------------
            with tc.tile_pool(name="psL", bufs=1, space="PSUM") as psL:
                pl = psL.tile([128, B], F32)
                for j in range(2):
                    for t in range(2):
                        nc.tensor.matmul(
                            pl[:, j * 512:(j + 1) * 512],
                            sqnT_sb[:, t],
                            msqnT_sb[:, t, j * 512:(j + 1) * 512],
                            start=(t == 0), stop=(t == 1))
                rsum = main.tile([128, 1], F32)
                pscr = scrp.tile([128, B], F16, tag="lscr")
                nc.scalar.activation(pscr[:], pl[:], AF.Exp,
                                     bias=bias_mx[:, 0:1],
                                     accum_out=rsum[:])
                nc.sync.dma_start(o_rowsum, rsum[:])
            box_casts(0)
            for g in (5, 7, 9, 11):
                nc.scalar.dma_start(bankT_sb[:, g], bankT[g])

            # ------------- score scan --------------------------------------
            with tc.tile_pool(name="psS", bufs=2, space="PSUM") as psS:
                for bt in range(NBT):
                    for pos, g in enumerate(SEQ):
                        ps = psS.tile([128, 2048], F32, tag="ps")
                        for k in range(4):
                            nc.tensor.matmul(
                                ps[:, k * 512:(k + 1) * 512],
                                nqT[:, :, bt * 128:(bt + 1) * 128],
                                bankT_sb[:, g, :, k * 512:(k + 1) * 512],
                                start=True, stop=True, perf_mode=DR)
                        if pos in DVE_POS[bt]:
                            nc.vector.tensor_reduce(
                                slot[:, bt, g],
                                ps[:].rearrange("p (s w) -> p s w", w=128),
                                axis=X, op=ALU.max)
                        else:
                            scr = scrp.tile([128, 2048], F16, tag="scr")
                            nc.scalar.activation(
                                scr[:], ps[:], AF.Exp,
                                bias=bias_sm[:, 0:1], scale=BETA,
                                accum_out=rs[:, bt, g:g + 1])
                        if bt == 0 and pos == 0:
                            box_tree(1)
                            box_casts(1)

                nc.sync.dma_start(
                    o_slot, slot[:].rearrange("p a b c -> p (a b c)"))
                nc.sync.dma_start(o_rs, rs[:].rearrange("p a b -> p (a b)"))

    nc.compile()
    return nc


def _get_module():
    if "nc" not in _CACHE:
        _CACHE["nc"] = _build_module()
    return _CACHE["nc"]


def _prep_inputs(inputs):
    import ml_dtypes
    f8 = ml_dtypes.float8_e4m3

    sq = np.asarray(inputs["s_query"], dtype=np.float32)
    msq = np.asarray(inputs["mem_s_query"], dtype=np.float32)
    box = np.asarray(inputs["s_box_feat"], dtype=np.float32).reshape(B, D, HWSP)
    bank = np.asarray(inputs["mem_bank"], dtype=np.float32)

    # normalized logits operands, pre-transposed, k-tile-major
    an = sq / np.maximum(np.linalg.norm(sq, axis=1, keepdims=True), 1e-12)
    cn = msq / np.maximum(np.linalg.norm(msq, axis=1, keepdims=True), 1e-12)
    anT = np.ascontiguousarray(
        (an / TEMP).T.reshape(2, 128, B)).astype(np.float16)       # [t,d',b]
    cnT = np.ascontiguousarray(cn.T.reshape(2, 128, B)).astype(np.float16)

    # box: [B, D, 49] -> fold h 49->48 -> [t, d', b, 48] f16
    box48 = box[:, :, :HF].copy()
    box48[:, :, 0] += box[:, :, HF]
    boxT = np.ascontiguousarray(
        box48.transpose(1, 0, 2).reshape(2, 128, B, HF)).astype(f8)

    # bank: [M, D] -> chunk-major DoubleRow layout [chunk, 128, 2, 2048] fp8
    bankT = np.ascontiguousarray(
        bank.T.reshape(2, 128, GM * NGRP, 2048).transpose(2, 1, 0, 3)
    ).astype(f8)

    in_maps = []
    for c in range(NCORES):
        g, h = c % GB, c // GB
        in_maps.append({
            "sqnT": np.ascontiguousarray(anT[:, :, 128 * c:128 * (c + 1)]),
            "msqnT": cnT,
            "boxT": np.ascontiguousarray(boxT[:, :, RB * g:RB * (g + 1), :]),
            "bankT": np.ascontiguousarray(bankT[NGRP * h:NGRP * (h + 1)]),
        })
    return in_maps, an, cn


def _finalize(results, an, cn):
    # per-row candidate merge (values are t' = -QSC*score)
    slot = np.stack([np.asarray(r["o_slot"], np.float32).reshape(
        128, NBT, NGRP, 16) for r in results])          # [8,128,2,16,16]
    rsv = np.stack([np.asarray(r["o_rs"], np.float64).reshape(
        128, NBT, NGRP) for r in results])              # [8,128,2,16]
    rowsum = np.concatenate(
        [np.asarray(r["o_rowsum"], np.float64)[:, 0] for r in results])

    with np.errstate(divide="ignore"):
        soft = C1 + np.log(rsv) / BETA                  # soft max of t'
    soft = np.where(np.isinf(soft) & (soft > 0), 100.0, soft)

    negsum = np.empty(B)
    for g in range(GB):
        cores = [g, g + GB]                             # the two M-halves
        for bt in range(NBT):
            cand = np.concatenate(
                [slot[ci][:, bt, DVE_GRPS[bt], :].reshape(128, -1)
                 for ci in cores]
                + [soft[ci][:, bt, SCL_GRPS[bt]] for ci in cores], axis=1)
            top5 = np.partition(cand, cand.shape[1] - 5, axis=1)[:, -5:]
            b0 = RB * g + 128 * bt
            negsum[b0:b0 + 128] = np.exp(
                -top5.astype(np.float64) / QSC).sum(axis=1)

    diag = (np.einsum("ij,ij->i", an, cn).astype(np.float32)
            / np.float32(TEMP)).astype(np.float64)
    loss_i = np.log(rowsum + np.exp(-MX) * negsum) - (diag - MX)
    m = loss_i.mean()
    if np.isnan(m):
        m = 0.0
    return np.float32(m)


def run(inputs, trace=False, **spmd_kwargs):
    from concourse.bass_utils import run_bass_kernel_spmd
    nc = _get_module()
    in_maps, an, cn = _prep_inputs(inputs)
    res = run_bass_kernel_spmd(nc, in_maps, core_ids=list(range(NCORES)),
                               trace=trace, **spmd_kwargs)
    loss = _finalize(res.results, an, cn)
    return loss, res


def kernel(**inputs) -> np.ndarray:
    loss, _ = run(inputs, trace=False)
    return loss


# revision 18
# speedup vs baseline: 1.0580x; 1.0580x over previous
"""Trainium2 Bass kernel for nn_MemConLoss_trans (supervised-contrastive loss
with memory-bank hard negatives).

Strategy (8 NeuronCores, SPMD, no collectives):
  - Grid-shard the score matrix: 4-way over B x 2-way over M. Core c handles
    score rows 256*(c%4) .. +256 against bank cols 32768*(c//4) .. +32768.
    Every tensor a core needs is staged from the host, so there is no
    AllGather (the baseline's collective cost ~96us of launch-skew barrier).
  - bank is host-pretransposed into the fp8 DoubleRow matmul layout
    [chunk, 128, 2, 2048]; score matmuls run in fp8 DoubleRow mode
    (2 rows/cycle, K=256 per instruction, ~157 TF/s when ramped).
  - box (s_box_feat) is host-cast to f16 with the h=48 column pre-folded
    (49 -> 48 spatial positions) and pretransposed to [d-tile, b, h]; the
    device reduces over h with a tensor_tensor pairwise fold tree (DVE 2x
    f16 mode, ~2x faster than tensor_reduce which has no fast mode), then
    the scalar engine casts to fp8 with the -4/49 mean+negate+range scale.
  - The -4*score matrix (8.4M f32 in PSUM per core) is scanned by both
    engines in parallel: DVE groups do a two-level tensor_tensor max fold
    (each tt reads two operands per cycle, so 2048 PSUM values cost ~1.3us)
    producing 512 slot-maxima shipped straight to DRAM; scalar groups do an
    Exp softmax-accumulator (one pass, accum_out) producing a soft max per
    2048 columns. Host merges both candidate kinds and takes the top-5
    smallest scores (their loss contribution is ~1e-6, so soft-max bias and
    fp8/f16 rounding are harmless).
  - The [B,B] contrastive logits are data-parallel over B (128 rows/core):
    host pre-normalizes and pre-transposes the operands, the device runs
    f32r matmuls and one exp row-sum on the scalar engine.
  - Host finalize: logits diagonal, candidate merge, final log/mean in f64.
    The constant shift 4.0 stands in for the per-row logits max (its only
    effect is ~1e-6 via exp(-MX)*negsum, as in the previous baseline).
"""

import numpy as np

B = 1024
D = 256
HWSP = 49              # 7*7 spatial positions
HF = 48                # h after host pre-fold of the odd column
NCORES = 8
GB = 4                 # B-shard factor of the score grid
GM = 2                 # M-shard factor
RB = B // GB           # 256 score rows per core
MC = 65536 // GM       # 32768 bank cols per core
NGRP = MC // 2048      # 16 psum groups per b-tile
NBT = RB // 128        # 2 b-tiles per core
MX = 4.0               # constant stand-in for the per-row logits max
TEMP = 0.07
QSC = 4.0              # fp8 range scale on the query
BETA = 2.0             # softmax sharpness on t' = -QSC*score
C1 = 40.0              # softmax shift on t'

# scan schedule: b-tile 0 consumes chunks as they arrive; once half of bt0
# is done, interleave b-tile-1 groups (SBUF-resident chunks) between bt0
# groups so chunk-delivery gaps are filled with reuse work.
ORDER = ([(0, g) for g in range(8)]
         + [p for k in range(8) for p in ((0, 8 + k), (1, k))]
         + [(1, 8 + k) for k in range(8)])
# groups consumed by the DVE slot-max lane (rest: scalar softmax-accum),
# chosen so consecutive ORDER entries alternate engines where possible
DVE_GRPS = [sorted({1, 3, 5, 7, 9, 11}), sorted({0, 2, 4, 6, 9, 11, 13})]
SCL_GRPS = [sorted(set(range(NGRP)) - set(DVE_GRPS[bt]))
            for bt in range(NBT)]

_CACHE = {}


def _build_module():
    import concourse.bacc as bacc
    import concourse.mybir as mybir
    import concourse.tile as tile

    F32 = mybir.dt.float32
    F32R = mybir.dt.float32r
    F16 = mybir.dt.float16
    FP8 = mybir.dt.float8e4
    AF = mybir.ActivationFunctionType
    ALU = mybir.AluOpType
    X = mybir.AxisListType.X
    DR = mybir.MatmulPerfMode.DoubleRow

    nc = bacc.Bacc("TRN2", target_bir_lowering=False, debug=False,
                   enable_asserts=False, num_devices=NCORES)

    sqnT = nc.dram_tensor("sqnT", [2, 128, 128], F16, kind="ExternalInput").ap()
    msqnT = nc.dram_tensor("msqnT", [2, 128, B], F16, kind="ExternalInput").ap()
    boxT = nc.dram_tensor("boxT", [2, 128, RB, HF], F16,
                          kind="ExternalInput").ap()
    bankT = nc.dram_tensor("bankT", [NGRP, 128, 2, 2048], FP8,
                           kind="ExternalInput").ap()
    o_rowsum = nc.dram_tensor("o_rowsum", [128, 1], F32,
                              kind="ExternalOutput").ap()
    o_slot = nc.dram_tensor("o_slot", [128, NBT * NGRP * 16], F16,
                            kind="ExternalOutput").ap()
    o_rs = nc.dram_tensor("o_rs", [128, NBT * NGRP], F32,
                          kind="ExternalOutput").ap()

    with nc.allow_low_precision("f16 box sums only feed the fp8 score matmul"):
      with tile.TileContext(nc) as tc:
        with (
            tc.tile_pool(name="main", bufs=1) as main,
            tc.tile_pool(name="scr", bufs=2) as scrp,
        ):
            sqnT_sb = main.tile([128, 2, 128], F16)
            msqnT_sb = main.tile([128, 2, B], F16)
            box_sb = main.tile([128, 2, RB, HF], F16)
            bankT_sb = main.tile([128, NGRP, 2, 2048], FP8)

            # ------------- input DMAs over 3 queues, arrival-ordered -------
            # act queue moves fastest early: give it box b-half 0 (the
            # critical path). gpsimd: logits operands then box q1 + odd tail.
            # sync: the bank even chunks. Late act DMAs are issued from
            # inside the scalar stream to dodge queue-credit blocking.
            Q = RB // 4  # 64-row box quarters

            def box_dma(eng, t, q):
                eng.dma_start(box_sb[:, t, q * Q:(q + 1) * Q, :],
                              boxT[t, :, q * Q:(q + 1) * Q, :])

            box_dma(nc.scalar, 0, 0)
            box_dma(nc.sync, 1, 0)
            box_dma(nc.scalar, 1, 1)
            nc.scalar.dma_start(bankT_sb[:, 0], bankT[0])
            box_dma(nc.gpsimd, 0, 1)
            nc.gpsimd.dma_start(msqnT_sb[:, 0], msqnT[0])
            nc.gpsimd.dma_start(msqnT_sb[:, 1], msqnT[1])
            nc.gpsimd.dma_start(sqnT_sb[:, 0], sqnT[0])
            nc.gpsimd.dma_start(sqnT_sb[:, 1], sqnT[1])
            nc.gpsimd.dma_start(bankT_sb[:, 1], bankT[1])
            for g in range(2, 16, 2):
                nc.sync.dma_start(bankT_sb[:, g], bankT[g])
            for g in (3, 5):
                nc.gpsimd.dma_start(bankT_sb[:, g], bankT[g])

            # ------------- box fold tree -> fp8 query ----------------------
            t24 = main.tile([128, 2, RB, 24], F16)
            t12 = main.tile([128, 2, RB, 12], F16)
            qsA = main.tile([128, 2, RB], F16)
            qsumT = main.tile([128, 2, RB], F16)
            nqT = main.tile([128, 2, RB], FP8)
            bias_mx = main.tile([128, 1], F32)
            nc.vector.memset(bias_mx[:], -MX)
            bias_sm = main.tile([128, 1], F32)
            nc.vector.memset(bias_sm[:], -BETA * C1)
            rs = main.tile([128, NBT, NGRP], F32)
            nc.vector.memset(rs[:], 0.0)
            slot = main.tile([128, NBT, NGRP, 16], F16)
            nc.vector.memset(slot[:], -1000.0)

            def box_casts(h):
                r = slice(h * 128, (h + 1) * 128)
                for t in range(2):
                    nc.scalar.activation(nqT[:, t, r], qsumT[:, t, r],
                                         AF.Copy, scale=-QSC / HWSP)

            def box_tree(h):  # h in {0,1}: rows h*128 .. h*128+128
                r = slice(h * 128, (h + 1) * 128)
                tt = nc.vector.tensor_tensor
                for q in (2 * h, 2 * h + 1):
                    s = slice(q * Q, (q + 1) * Q)
                    tt(out=t24[:, :, s, :], in0=box_sb[:, :, s, 0:24],
                       in1=box_sb[:, :, s, 24:48], op=ALU.add)
                tt(out=t12[:, :, r, :], in0=t24[:, :, r, 0:12],
                   in1=t24[:, :, r, 12:24], op=ALU.add)
                tt(out=t24[:, :, r, 0:6], in0=t12[:, :, r, 0:6],
                   in1=t12[:, :, r, 6:12], op=ALU.add)
                tt(out=t12[:, :, r, 0:3], in0=t24[:, :, r, 0:3],
                   in1=t24[:, :, r, 3:6], op=ALU.add)
                tt(out=qsA[:, :, r], in0=t12[:, :, r, 0],
                   in1=t12[:, :, r, 1], op=ALU.add)
                tt(out=qsumT[:, :, r], in0=qsA[:, :, r],
                   in1=t12[:, :, r, 2], op=ALU.add)

            box_tree(0)
            for q in (2, 3):
                box_dma(nc.scalar, 0, q)
                box_dma(nc.scalar, 1, q)
            for g in (7, 9, 11, 13, 15):
                nc.gpsimd.dma_start(bankT_sb[:, g], bankT[g])

            # ------------- logits: f32r matmul + exp rowsum ----------------
            with tc.tile_pool(name="psL", bufs=1, space="PSUM") as psL:
                pl = psL.tile([128, B], F32)
                for j in range(2):
                    for t in range(2):
                        nc.tensor.matmul(
                            pl[:, j * 512:(j + 1) * 512],
                            sqnT_sb[:, t],
                            msqnT_sb[:, t, j * 512:(j + 1) * 512],
                            start=(t == 0), stop=(t == 1))
                rsum = main.tile([128, 1], F32)
                pscr = scrp.tile([128, B], F16, tag="lscr")
                nc.scalar.activation(pscr[:], pl[:], AF.Exp,
                                     bias=bias_mx[:, 0:1],
                                     accum_out=rsum[:])
                nc.sync.dma_start(o_rowsum, rsum[:])
            box_casts(0)

            # ------------- score scan --------------------------------------
            dve_sets = [set(DVE_GRPS[bt]) for bt in range(NBT)]
            with tc.tile_pool(name="psS", bufs=2, space="PSUM") as psS:
                for pos, (bt, g) in enumerate(ORDER):
                        ps = psS.tile([128, 2048], F32, tag="ps")
                        for k in range(4):
                            nc.tensor.matmul(
                                ps[:, k * 512:(k + 1) * 512],
                                nqT[:, :, bt * 128:(bt + 1) * 128],
                                bankT_sb[:, g, :, k * 512:(k + 1) * 512],
                                start=True, stop=True, perf_mode=DR)
                        if g in dve_sets[bt]:
                            nc.vector.tensor_reduce(
                                slot[:, bt, g],
                                ps[:].rearrange("p (s w) -> p s w", w=128),
                                axis=X, op=ALU.max)
                        else:
                            scr = scrp.tile([128, 2048], F16, tag="scr")
                            nc.scalar.activation(
                                scr[:], ps[:], AF.Exp,
                                bias=bias_sm[:, 0:1], scale=BETA,
                                accum_out=rs[:, bt, g:g + 1])
                        if pos == 0:
                            box_tree(1)
                            box_casts(1)

                nc.sync.dma_start(
                    o_slot, slot[:].rearrange("p a b c -> p (a b c)"))
                nc.sync.dma_start(o_rs, rs[:].rearrange("p a b -> p (a b)"))

    nc.compile()
    return nc


def _get_module():
    if "nc" not in _CACHE:
        _CACHE["nc"] = _build_module()
    return _CACHE["nc"]


def _prep_inputs(inputs):
    import ml_dtypes
    f8 = ml_dtypes.float8_e4m3

    sq = np.asarray(inputs["s_query"], dtype=np.float32)
    msq = np.asarray(inputs["mem_s_query"], dtype=np.float32)
    box = np.asarray(inputs["s_box_feat"], dtype=np.float32).reshape(B, D, HWSP)
    bank = np.asarray(inputs["mem_bank"], dtype=np.float32)

    # normalized logits operands, pre-transposed, k-tile-major
    an = sq / np.maximum(np.linalg.norm(sq, axis=1, keepdims=True), 1e-12)
    cn = msq / np.maximum(np.linalg.norm(msq, axis=1, keepdims=True), 1e-12)
    anT = np.ascontiguousarray(
        (an / TEMP).T.reshape(2, 128, B)).astype(np.float16)       # [t,d',b]
    cnT = np.ascontiguousarray(cn.T.reshape(2, 128, B)).astype(np.float16)

    # box: [B, D, 49] -> fold h 49->48 -> [t, d', b, 48] f16
    box48 = box[:, :, :HF].copy()
    box48[:, :, 0] += box[:, :, HF]
    boxT = np.ascontiguousarray(
        box48.transpose(1, 0, 2).reshape(2, 128, B, HF)).astype(np.float16)

    # bank: [M, D] -> chunk-major DoubleRow layout [chunk, 128, 2, 2048] fp8
    bankT = np.ascontiguousarray(
        bank.T.reshape(2, 128, GM * NGRP, 2048).transpose(2, 1, 0, 3)
    ).astype(f8)

    in_maps = []
    for c in range(NCORES):
        g, h = c % GB, c // GB
        in_maps.append({
            "sqnT": np.ascontiguousarray(anT[:, :, 128 * c:128 * (c + 1)]),
            "msqnT": cnT,
            "boxT": np.ascontiguousarray(boxT[:, :, RB * g:RB * (g + 1), :]),
            "bankT": np.ascontiguousarray(bankT[NGRP * h:NGRP * (h + 1)]),
        })
    return in_maps, an, cn


def _finalize(results, an, cn):
    # per-row candidate merge (values are t' = -QSC*score)
    slot = np.stack([np.asarray(r["o_slot"], np.float32).reshape(
        128, NBT, NGRP, 16) for r in results])          # [8,128,2,16,16]
    rsv = np.stack([np.asarray(r["o_rs"], np.float64).reshape(
        128, NBT, NGRP) for r in results])              # [8,128,2,16]
    rowsum = np.concatenate(
        [np.asarray(r["o_rowsum"], np.float64)[:, 0] for r in results])

    with np.errstate(divide="ignore"):
        soft = C1 + np.log(rsv) / BETA                  # soft max of t'
    soft = np.where(np.isinf(soft) & (soft > 0), 100.0, soft)

    negsum = np.empty(B)
    for g in range(GB):
        cores = [g, g + GB]                             # the two M-halves
        for bt in range(NBT):
            cand = np.concatenate(
                [slot[ci][:, bt, DVE_GRPS[bt], :].reshape(128, -1)
                 for ci in cores]
                + [soft[ci][:, bt, SCL_GRPS[bt]] for ci in cores], axis=1)
            top5 = np.partition(cand, cand.shape[1] - 5, axis=1)[:, -5:]
            b0 = RB * g + 128 * bt
            negsum[b0:b0 + 128] = np.exp(
                -top5.astype(np.float64) / QSC).sum(axis=1)

    diag = (np.einsum("ij,ij->i", an, cn).astype(np.float32)
            / np.float32(TEMP)).astype(np.float64)
    loss_i = np.log(rowsum + np.exp(-MX) * negsum) - (diag - MX)
    m = loss_i.mean()
    if np.isnan(m):
        m = 0.0
    return np.float32(m)


def run(inputs, trace=False, **spmd_kwargs):
    from concourse.bass_utils import run_bass_kernel_spmd
    nc = _get_module()
    in_maps, an, cn = _prep_inputs(inputs)
    res = run_bass_kernel_spmd(nc, in_maps, core_ids=list(range(NCORES)),
                               trace=trace, **spmd_kwargs)
    loss = _finalize(res.results, an, cn)
    return loss, res


def kernel(**inputs) -> np.ndarray:
    loss, _ = run(inputs, trace=False)
    return loss


# revision 19
# speedup vs baseline: 1.0917x; 1.0318x over previous
"""Trainium2 Bass kernel for nn_MemConLoss_trans (supervised-contrastive loss
with memory-bank hard negatives).

Strategy (8 NeuronCores, SPMD, no collectives):
  - Grid-shard the score matrix: 4-way over B x 2-way over M. Core c handles
    score rows 256*(c%4) .. +256 against bank cols 32768*(c//4) .. +32768.
    Every tensor a core needs is staged from the host, so there is no
    AllGather (the baseline's collective cost ~96us of launch-skew barrier).
  - bank is host-pretransposed into the fp8 DoubleRow matmul layout
    [chunk, 128, 2, 2048]; score matmuls run in fp8 DoubleRow mode
    (2 rows/cycle, K=256 per instruction, ~157 TF/s when ramped).
  - box (s_box_feat) is host-cast to f16 with the h=48 column pre-folded
    (49 -> 48 spatial positions) and pretransposed to [d-tile, b, h]; the
    device reduces over h with a tensor_tensor pairwise fold tree (DVE 2x
    f16 mode, ~2x faster than tensor_reduce which has no fast mode), then
    the scalar engine casts to fp8 with the -4/49 mean+negate+range scale.
  - The -4*score matrix (8.4M f32 in PSUM per core) is scanned by both
    engines in parallel: DVE groups do a two-level tensor_tensor max fold
    (each tt reads two operands per cycle, so 2048 PSUM values cost ~1.3us)
    producing 512 slot-maxima shipped straight to DRAM; scalar groups do an
    Exp softmax-accumulator (one pass, accum_out) producing a soft max per
    2048 columns. Host merges both candidate kinds and takes the top-5
    smallest scores (their loss contribution is ~1e-6, so soft-max bias and
    fp8/f16 rounding are harmless).
  - The [B,B] contrastive logits are data-parallel over B (128 rows/core):
    host pre-normalizes and pre-transposes the operands, the device runs
    f32r matmuls and one exp row-sum on the scalar engine.
  - Host finalize: logits diagonal, candidate merge, final log/mean in f64.
    The constant shift 4.0 stands in for the per-row logits max (its only
    effect is ~1e-6 via exp(-MX)*negsum, as in the previous baseline).
"""

import numpy as np

B = 1024
D = 256
HWSP = 49              # 7*7 spatial positions
HF = 48                # h after host pre-fold of the odd column
NCORES = 8
GB = 4                 # B-shard factor of the score grid
GM = 2                 # M-shard factor
RB = B // GB           # 256 score rows per core
MC = 65536 // GM       # 32768 bank cols per core
NGRP = MC // 2048      # 16 psum groups per b-tile
NBT = RB // 128        # 2 b-tiles per core
MX = 4.0               # constant stand-in for the per-row logits max
TEMP = 0.07
QSC = 4.0              # fp8 range scale on the query
BETA = 2.0             # softmax sharpness on t' = -QSC*score
C1 = 40.0              # softmax shift on t'

# scan order: chunks in DMA-arrival order (sync evens, act odds, gpsimd tail)
SEQ = list(range(16))
# positions in SEQ consumed by the DVE slot-max lane; scalar takes the rest
DVE_POS = [{1, 3, 5, 7, 9, 11}, {1, 3, 5, 7, 9, 11, 13}]
DVE_GRPS = [sorted(SEQ[p] for p in DVE_POS[bt]) for bt in range(NBT)]
SCL_GRPS = [sorted(SEQ[p] for p in set(range(NGRP)) - DVE_POS[bt])
            for bt in range(NBT)]

_CACHE = {}


def _build_module():
    import concourse.bacc as bacc
    import concourse.mybir as mybir
    import concourse.tile as tile

    F32 = mybir.dt.float32
    F32R = mybir.dt.float32r
    F16 = mybir.dt.float16
    FP8 = mybir.dt.float8e4
    AF = mybir.ActivationFunctionType
    ALU = mybir.AluOpType
    X = mybir.AxisListType.X
    DR = mybir.MatmulPerfMode.DoubleRow

    nc = bacc.Bacc("TRN2", target_bir_lowering=False, debug=False,
                   enable_asserts=False, num_devices=NCORES)

    sqnT = nc.dram_tensor("sqnT", [2, 128, 128], F16, kind="ExternalInput").ap()
    msqnT = nc.dram_tensor("msqnT", [2, 128, B], F16, kind="ExternalInput").ap()
    boxT = nc.dram_tensor("boxT", [2, 128, RB, HF], F16,
                          kind="ExternalInput").ap()
    bankT = nc.dram_tensor("bankT", [NGRP, 128, 2, 2048], FP8,
                           kind="ExternalInput").ap()
    o_rowsum = nc.dram_tensor("o_rowsum", [128, 1], F32,
                              kind="ExternalOutput").ap()
    o_slot = nc.dram_tensor("o_slot", [128, NBT * NGRP * 16], F16,
                            kind="ExternalOutput").ap()
    o_rs = nc.dram_tensor("o_rs", [128, NBT * NGRP], F32,
                          kind="ExternalOutput").ap()

    with nc.allow_low_precision("f16 box sums only feed the fp8 score matmul"):
      with tile.TileContext(nc) as tc:
        with (
            tc.tile_pool(name="main", bufs=1) as main,
            tc.tile_pool(name="scr", bufs=2) as scrp,
        ):
            sqnT_sb = main.tile([128, 2, 128], F16)
            msqnT_sb = main.tile([128, 2, B], F16)
            box_sb = main.tile([128, 2, RB, HF], F16)
            bankT_sb = main.tile([128, NGRP, 2, 2048], FP8)

            # ------------- input DMAs over 3 queues, arrival-ordered -------
            # act queue moves fastest early: give it box b-half 0 (the
            # critical path). gpsimd: logits operands then box q1 + odd tail.
            # sync: the bank even chunks. Late act DMAs are issued from
            # inside the scalar stream to dodge queue-credit blocking.
            Q = RB // 4  # 64-row box quarters

            def box_dma(eng, t, q):
                eng.dma_start(box_sb[:, t, q * Q:(q + 1) * Q, :],
                              boxT[t, :, q * Q:(q + 1) * Q, :])

            box_dma(nc.scalar, 0, 0)
            box_dma(nc.sync, 1, 0)
            box_dma(nc.scalar, 1, 1)
            nc.scalar.dma_start(bankT_sb[:, 0], bankT[0])
            box_dma(nc.gpsimd, 0, 1)
            nc.gpsimd.dma_start(msqnT_sb[:, 0], msqnT[0])
            nc.gpsimd.dma_start(msqnT_sb[:, 1], msqnT[1])
            nc.gpsimd.dma_start(sqnT_sb[:, 0], sqnT[0])
            nc.gpsimd.dma_start(sqnT_sb[:, 1], sqnT[1])
            nc.gpsimd.dma_start(bankT_sb[:, 1], bankT[1])
            for g in range(2, 16, 2):
                nc.sync.dma_start(bankT_sb[:, g], bankT[g])
            for g in (3, 5):
                nc.gpsimd.dma_start(bankT_sb[:, g], bankT[g])

            # ------------- box fold tree -> fp8 query ----------------------
            t24 = main.tile([128, 2, RB, 24], F16)
            t12 = main.tile([128, 2, RB, 12], F16)
            qsA = main.tile([128, 2, RB], F16)
            qsumT = main.tile([128, 2, RB], F16)
            nqT = main.tile([128, 2, RB], FP8)
            bias_mx = main.tile([128, 1], F32)
            nc.vector.memset(bias_mx[:], -MX)
            bias_sm = main.tile([128, 1], F32)
            nc.vector.memset(bias_sm[:], -BETA * C1)
            rs = main.tile([128, NBT, NGRP], F32)
            nc.vector.memset(rs[:], 0.0)
            slot = main.tile([128, NBT, NGRP, 16], F16)
            nc.vector.memset(slot[:], -1000.0)

            def box_casts(h):
                r = slice(h * 128, (h + 1) * 128)
                for t in range(2):
                    nc.scalar.activation(nqT[:, t, r], qsumT[:, t, r],
                                         AF.Copy, scale=-QSC / HWSP)

            def box_tree(h):  # h in {0,1}: rows h*128 .. h*128+128
                r = slice(h * 128, (h + 1) * 128)
                tt = nc.vector.tensor_tensor
                for q in (2 * h, 2 * h + 1):
                    s = slice(q * Q, (q + 1) * Q)
                    tt(out=t24[:, :, s, :], in0=box_sb[:, :, s, 0:24],
                       in1=box_sb[:, :, s, 24:48], op=ALU.add)
                tt(out=t12[:, :, r, :], in0=t24[:, :, r, 0:12],
                   in1=t24[:, :, r, 12:24], op=ALU.add)
                tt(out=t24[:, :, r, 0:6], in0=t12[:, :, r, 0:6],
                   in1=t12[:, :, r, 6:12], op=ALU.add)
                tt(out=t12[:, :, r, 0:3], in0=t24[:, :, r, 0:3],
                   in1=t24[:, :, r, 3:6], op=ALU.add)
                tt(out=qsA[:, :, r], in0=t12[:, :, r, 0],
                   in1=t12[:, :, r, 1], op=ALU.add)
                tt(out=qsumT[:, :, r], in0=qsA[:, :, r],
                   in1=t12[:, :, r, 2], op=ALU.add)

            box_tree(0)
            for q in (2, 3):
                box_dma(nc.scalar, 0, q)
                box_dma(nc.scalar, 1, q)
            for g in (7, 9, 11, 13, 15):
                nc.gpsimd.dma_start(bankT_sb[:, g], bankT[g])

            # ------------- logits: f32r matmul + exp rowsum ----------------
            with tc.tile_pool(name="psL", bufs=1, space="PSUM") as psL:
                pl = psL.tile([128, B], F32)
                for j in range(2):
                    for t in range(2):
                        nc.tensor.matmul(
                            pl[:, j * 512:(j + 1) * 512],
                            sqnT_sb[:, t],
                            msqnT_sb[:, t, j * 512:(j + 1) * 512],
                            start=(t == 0), stop=(t == 1))
                rsum = main.tile([128, 1], F32)
                pscr = scrp.tile([128, B], F16, tag="lscr")
                nc.scalar.activation(pscr[:], pl[:], AF.Exp,
                                     bias=bias_mx[:, 0:1],
                                     accum_out=rsum[:])
                nc.sync.dma_start(o_rowsum, rsum[:])
            box_casts(0)

            # ------------- score scan --------------------------------------
            with tc.tile_pool(name="psS", bufs=2, space="PSUM") as psS:
                for bt in range(NBT):
                    for pos, g in enumerate(SEQ):
                        ps = psS.tile([128, 2048], F32, tag="ps")
                        for k in range(4):
                            nc.tensor.matmul(
                                ps[:, k * 512:(k + 1) * 512],
                                nqT[:, :, bt * 128:(bt + 1) * 128],
                                bankT_sb[:, g, :, k * 512:(k + 1) * 512],
                                start=True, stop=True, perf_mode=DR)
                        if pos in DVE_POS[bt]:
                            nc.vector.tensor_reduce(
                                slot[:, bt, g],
                                ps[:].rearrange("p (s w) -> p s w", w=128),
                                axis=X, op=ALU.max)
                        else:
                            scr = scrp.tile([128, 2048], F16, tag="scr")
                            nc.scalar.activation(
                                scr[:], ps[:], AF.Exp,
                                bias=bias_sm[:, 0:1], scale=BETA,
                                accum_out=rs[:, bt, g:g + 1])
                        if bt == 0 and pos == 0:
                            box_tree(1)
                            box_casts(1)

                nc.sync.dma_start(
                    o_slot, slot[:].rearrange("p a b c -> p (a b c)"))
                nc.sync.dma_start(o_rs, rs[:].rearrange("p a b -> p (a b)"))

    nc.compile()
    return nc


def _get_module():
    if "nc" not in _CACHE:
        _CACHE["nc"] = _build_module()
    return _CACHE["nc"]


def _prep_inputs(inputs):
    import ml_dtypes
    f8 = ml_dtypes.float8_e4m3

    sq = np.asarray(inputs["s_query"], dtype=np.float32)
    msq = np.asarray(inputs["mem_s_query"], dtype=np.float32)
    box = np.asarray(inputs["s_box_feat"], dtype=np.float32).reshape(B, D, HWSP)
    bank = np.asarray(inputs["mem_bank"], dtype=np.float32)

    # normalized logits operands, pre-transposed, k-tile-major
    an = sq / np.maximum(np.linalg.norm(sq, axis=1, keepdims=True), 1e-12)
    cn = msq / np.maximum(np.linalg.norm(msq, axis=1, keepdims=True), 1e-12)
    anT = np.ascontiguousarray(
        (an / TEMP).T.reshape(2, 128, B)).astype(np.float16)       # [t,d',b]
    cnT = np.ascontiguousarray(cn.T.reshape(2, 128, B)).astype(np.float16)

    # box: [B, D, 49] -> fold h 49->48 -> [t, d', b, 48] f16
    box48 = box[:, :, :HF].copy()
    box48[:, :, 0] += box[:, :, HF]
    boxT = np.ascontiguousarray(
        box48.transpose(1, 0, 2).reshape(2, 128, B, HF)).astype(np.float16)

    # bank: [M, D] -> chunk-major DoubleRow layout [chunk, 128, 2, 2048] fp8
    bankT = np.ascontiguousarray(
        bank.T.reshape(2, 128, GM * NGRP, 2048).transpose(2, 1, 0, 3)
    ).astype(f8)

    in_maps = []
    for c in range(NCORES):
        g, h = c % GB, c // GB
        in_maps.append({
            "sqnT": np.ascontiguousarray(anT[:, :, 128 * c:128 * (c + 1)]),
            "msqnT": cnT,
            "boxT": np.ascontiguousarray(boxT[:, :, RB * g:RB * (g + 1), :]),
            "bankT": np.ascontiguousarray(bankT[NGRP * h:NGRP * (h + 1)]),
        })
    return in_maps, an, cn


def _finalize(results, an, cn):
    # per-row candidate merge (values are t' = -QSC*score)
    slot = np.stack([np.asarray(r["o_slot"], np.float32).reshape(
        128, NBT, NGRP, 16) for r in results])          # [8,128,2,16,16]
    rsv = np.stack([np.asarray(r["o_rs"], np.float64).reshape(
        128, NBT, NGRP) for r in results])              # [8,128,2,16]
    rowsum = np.concatenate(
        [np.asarray(r["o_rowsum"], np.float64)[:, 0] for r in results])

    with np.errstate(divide="ignore"):
        soft = C1 + np.log(rsv) / BETA                  # soft max of t'
    soft = np.where(np.isinf(soft) & (soft > 0), 100.0, soft)

    negsum = np.empty(B)
    for g in range(GB):
        cores = [g, g + GB]                             # the two M-halves
        for bt in range(NBT):
            cand = np.concatenate(
                [slot[ci][:, bt, DVE_GRPS[bt], :].reshape(128, -1)
                 for ci in cores]
                + [soft[ci][:, bt, SCL_GRPS[bt]] for ci in cores], axis=1)
            top5 = np.partition(cand, cand.shape[1] - 5, axis=1)[:, -5:]
            b0 = RB * g + 128 * bt
            negsum[b0:b0 + 128] = np.exp(
                -top5.astype(np.float64) / QSC).sum(axis=1)

    diag = (np.einsum("ij,ij->i", an, cn).astype(np.float32)
            / np.float32(TEMP)).astype(np.float64)
    loss_i = np.log(rowsum + np.exp(-MX) * negsum) - (diag - MX)
    m = loss_i.mean()
    if np.isnan(m):
        m = 0.0
    return np.float32(m)


def run(inputs, trace=False, **spmd_kwargs):
    from concourse.bass_utils import run_bass_kernel_spmd
    nc = _get_module()
    in_maps, an, cn = _prep_inputs(inputs)
    res = run_bass_kernel_spmd(nc, in_maps, core_ids=list(range(NCORES)),
                               trace=trace, **spmd_kwargs)
    loss = _finalize(res.results, an, cn)
    return loss, res


def kernel(**inputs) -> np.ndarray:
    loss, _ = run(inputs, trace=False)
    return loss
